# revision 15
# baseline (speedup 1.0000x reference)
"""Block-diagonal linear y = x @ W_blockdiag.T + bias on 8 TRN2 NeuronCores.

Expert-parallel sharding: core k owns diagonal block k — x[:, 512k:512(k+1)],
weight_blocks[k] (512x512), bias[512k:512(k+1)] — and produces the matching
output column slice y[:, 512k:512(k+1)]. No collectives.

Per-core kernel (Tile framework):
  - load x in 2 MiB chunks (8 token-tiles of [128, 512] per DMA)
  - PE-transpose each [128,128] sub-block of an x tile into PSUM (exact f32),
    evacuate as one [128, 512] strip to SBUF -> xT blocks [c=128, n=128]
  - 4 accumulating matmuls per token tile: stationary lhsT = xT block,
    moving rhs = W.T strip [c=128, r=512], dtype float32r (1 cyc/row)
  - bias add fused into the PSUM->SBUF evacuation on DVE
  - store y in 2 MiB chunks
"""

import os
import sys

import numpy as np

for _p in ("/opt/trn_rl_repo", "/root/.axon_site/_ro/trn_rl_repo"):
    if os.path.isdir(_p) and _p not in sys.path:
        sys.path.insert(0, _p)

import concourse.bass as bass
import concourse.mybir as mybir
import concourse.tile as tile
from concourse.masks import make_identity
from concourse.bass_utils import run_bass_kernel_spmd
from concourse.tile_rust import add_dep_helper

# Problem shape (hardcoded per spec nn_BlockDiagLinear_19490561590005)
N = 8192          # tokens
D = 4096          # model dim
NB = 8            # diagonal blocks == number of cores
B = 512           # block size (rows == cols)
P = 128           # SBUF partitions
CB = B // P       # 4 contraction chunks of 128
G = 8             # token tiles per DMA chunk (2 MiB per transfer)
NT = N // P       # 64 token tiles
NCHUNK = NT // G  # 8 DMA chunks

F32 = mybir.dt.float32
# float32r: 1 cycle/row on the PE for free dim >= 256 (vs 4 for float32)
MM_DT = getattr(mybir.dt, os.environ.get("BD_MM_DT", "float32r"))

_CACHE = {}


def _build_bass():
    nc = bass.Bass("TRN2", target_bir_lowering=False)
    x_d = nc.dram_tensor("x", [N, B], F32, kind="ExternalInput")
    w_d = nc.dram_tensor("w", [B, B], F32, kind="ExternalInput")
    b_d = nc.dram_tensor("b", [B], F32, kind="ExternalInput")
    y_d = nc.dram_tensor("y", [N, B], F32, kind="ExternalOutput")

    with tile.TileContext(nc) as tc:
        with (
            tc.tile_pool(name="const", bufs=1) as const_pool,
            tc.tile_pool(name="xin", bufs=2) as x_pool,
            tc.tile_pool(name="yout", bufs=2) as y_pool,
            tc.tile_pool(name="xT", bufs=3) as xT_pool,
            tc.tile_pool(name="psT", bufs=3, space="PSUM") as psT_pool,
            tc.tile_pool(name="psY", bufs=4, space="PSUM") as psY_pool,
            tc.tile_pool(name="psDummy", bufs=1, space="PSUM") as psD_pool,
        ):
            ident = const_pool.tile([P, P], F32)
            make_identity(nc, ident)

            # Wait-staggering: the self-loading (fp32/fp32r) matmul lowering
            # supports only one attached sync wait. The first real PE
            # instruction would otherwise wait on both the identity (Pool sem)
            # and a DMA sem. Absorb the identity wait with a dummy transpose
            # that reads only the identity tile.
            ps_dummy = psD_pool.tile([P, P], F32)
            dummy_inst = nc.tensor.transpose(ps_dummy, ident, ident)

            # bias replicated to all 128 partitions: [128, 512]
            bias_rep = const_pool.tile([P, B], F32)
            nc.sync.dma_start(
                out=bias_rep,
                in_=b_d.ap().unsqueeze(0).partition_broadcast(P),
            )

            # W natural layout: 4 row-blocks side by side -> [128, 4*512]
            # w_nat[:, rj*512:(rj+1)*512] = W[rj*128:(rj+1)*128, :]
            w_nat = const_pool.tile([P, CB * B], F32)
            nc.sync.dma_start(
                out=w_nat.rearrange("p (j c) -> p j c", j=CB),
                in_=w_d.ap().rearrange("(j p) c -> p j c", p=P),
            )

            # wT strips: wT[:, ci*512 + r] (c on partitions) = W[r, ci*128 + c]
            # dtype MM_DT: the PSUM->SBUF copy rounds to fp32r as the BIR
            # verifier requires for fp32r matmul operands
            wT = const_pool.tile([P, CB * B], MM_DT)
            for ci in range(CB):
                psT = psT_pool.tile([P, B], F32, tag="ps_t")
                for rj in range(CB):
                    t_inst = nc.tensor.transpose(
                        psT[:, rj * P : (rj + 1) * P],
                        w_nat[:, rj * B + ci * P : rj * B + ci * P + P],
                        ident,
                    )
                    if ci == 0 and rj == 0:
                        add_dep_helper(
                            t_inst.ins, dummy_inst.ins, sync=False,
                            reason="stagger startup waits",
                        )
                nc.vector.tensor_copy(out=wT[:, ci * B : (ci + 1) * B], in_=psT)

            # main loop over 8 chunks of 1024 tokens
            for t in range(NCHUNK):
                x_big = x_pool.tile([P, G * B], F32)
                nc.sync.dma_start(
                    out=x_big.rearrange("p (g c) -> p g c", g=G),
                    in_=x_d.ap()[t * G * P : (t + 1) * G * P, :].rearrange(
                        "(g p) c -> p g c", p=P
                    ),
                )
                y_big = y_pool.tile([P, G * B], F32)
                for g in range(G):
                    xs = x_big[:, g * B : (g + 1) * B]  # [n=128, c=512]
                    psx = psT_pool.tile([P, B], F32, tag="ps_t")
                    for ci in range(CB):
                        t_inst = nc.tensor.transpose(
                            psx[:, ci * P : (ci + 1) * P],
                            xs[:, ci * P : (ci + 1) * P],
                            ident,
                        )
                        if t == 0 and g == 0 and ci == 0:
                            add_dep_helper(
                                t_inst.ins, dummy_inst.ins, sync=False,
                                reason="stagger startup waits",
                            )
                    xT = xT_pool.tile([P, B], MM_DT)
                    nc.vector.tensor_copy(out=xT, in_=psx)

                    psy = psY_pool.tile([P, B], F32)
                    for ci in range(CB):
                        nc.tensor.matmul(
                            psy,
                            xT[:, ci * P : (ci + 1) * P],
                            wT[:, ci * B : (ci + 1) * B],
                            start=(ci == 0),
                            stop=(ci == CB - 1),
                        )
                    # fused bias add + PSUM->SBUF evacuation
                    nc.vector.tensor_add(
                        y_big[:, g * B : (g + 1) * B], psy, bias_rep
                    )

                nc.sync.dma_start(
                    out=y_d.ap()[t * G * P : (t + 1) * G * P, :].rearrange(
                        "(g p) c -> p g c", p=P
                    ),
                    in_=y_big.rearrange("p (g c) -> p g c", g=G),
                )

    return nc


def _split_pe_multiwaits(nc):
    """Hoist extra sync waits off engine instructions onto sequencer NoOps.

    This walrus build supports only a single attached sync wait per
    instruction; codegen fails with "Too many sync wait commands" otherwise.
    A wait-carrying NoOp immediately before the instruction on the same
    sequencer is semantically identical (the sequencer executes in order).
    """
    k = 0
    for f in nc.m.functions:
        for blk in f.blocks:
            out = []
            changed = False
            for inst in blk.instructions:
                si = inst.sync_info
                if si is not None and len(si.on_wait) > 1:
                    waits = list(si.on_wait)
                    for w in waits[:-1]:
                        nop = mybir.InstNoOp(
                            name=f"I-waitsplit-{k}", ins=[], outs=[]
                        )
                        k += 1
                        nop.engine = inst.engine
                        nop.sync_info = mybir.SyncInfo(on_wait=[w], on_update=[])
                        out.append(nop)
                    inst.sync_info = mybir.SyncInfo(
                        on_wait=[waits[-1]], on_update=list(si.on_update)
                    )
                    changed = True
                out.append(inst)
            if changed:
                blk.instructions = out
    return nc


def _get_nc():
    if "nc" not in _CACHE:
        _CACHE["nc"] = _split_pe_multiwaits(_build_bass())
    return _CACHE["nc"]


def _run(inputs, trace=False):
    x = np.ascontiguousarray(np.asarray(inputs["x"], dtype=np.float32))
    w = np.ascontiguousarray(np.asarray(inputs["weight_blocks"], dtype=np.float32))
    bias = np.ascontiguousarray(np.asarray(inputs["bias"], dtype=np.float32))
    assert x.shape == (N, D) and w.shape == (NB, B, B) and bias.shape == (D,)

    nc = _get_nc()
    in_maps = [
        {
            "x": np.ascontiguousarray(x[:, k * B : (k + 1) * B]),
            "w": np.ascontiguousarray(w[k]),
            "b": np.ascontiguousarray(bias[k * B : (k + 1) * B]),
        }
        for k in range(NB)
    ]
    res = run_bass_kernel_spmd(nc, in_maps, core_ids=list(range(NB)), trace=trace)
    y = np.concatenate([res.results[k]["y"] for k in range(NB)], axis=1)
    return np.asarray(y, dtype=np.float32), res


def kernel(**inputs):
    y, _ = _run(inputs, trace=False)
    return y


def kernel_traced(**inputs):
    return _run(inputs, trace=True)


# revision 17
# speedup vs baseline: 1.0159x; 1.0159x over previous
"""Block-diagonal linear y = x @ W_blockdiag.T + bias on 8 TRN2 NeuronCores.

Expert-parallel sharding: core k owns diagonal block k — x[:, 512k:512(k+1)],
weight_blocks[k] (512x512), bias[512k:512(k+1)] — and produces the matching
output column slice y[:, 512k:512(k+1)]. No collectives.

Per-core kernel (Tile framework):
  - load x in staggered chunks (small first chunks for fast pipeline fill,
    2 MiB steady-state DMAs)
  - PE-transpose each [128,128] sub-block of an x tile into PSUM (float32r,
    1.5 cyc/row), evacuate as one [128, 512] strip to SBUF with a rounding
    cast -> xT blocks [c=128, n=128] in float32r
  - 4 accumulating matmuls per token tile: stationary lhsT = xT block,
    moving rhs = W.T strip [c=128, r=512], float32r (1 cyc/row)
  - bias add fused into the PSUM->SBUF evacuation on DVE
  - store y in staggered chunks (small last chunks to shorten the tail)
"""

import os
import sys

import numpy as np

for _p in ("/opt/trn_rl_repo", "/root/.axon_site/_ro/trn_rl_repo"):
    if os.path.isdir(_p) and _p not in sys.path:
        sys.path.insert(0, _p)

import concourse.bass as bass
import concourse.mybir as mybir
import concourse.tile as tile
from concourse.masks import make_identity
from concourse.bass_utils import run_bass_kernel_spmd
from concourse.tile_rust import add_dep_helper

# Problem shape (hardcoded per spec nn_BlockDiagLinear_19490561590005)
N = 8192          # tokens
D = 4096          # model dim
NB = 8            # diagonal blocks == number of cores
B = 512           # block size (rows == cols)
P = 128           # SBUF partitions
CB = B // P       # 4 contraction chunks of 128
NT = N // P       # 64 token tiles

F32 = mybir.dt.float32
# float32r: 1 cycle/row on the PE for free dim >= 256 (vs 4 for float32)
MM_DT = getattr(mybir.dt, os.environ.get("BD_MM_DT", "float32r"))

# token tiles per DMA: small first x chunks (fast pipeline fill), small
# last y chunks (short tail)
X_SCHED = [2, 2, 4] + [8] * 7
Y_SCHED = [8] * 7 + [4, 2, 2]
assert sum(X_SCHED) == NT and sum(Y_SCHED) == NT

_CACHE = {}


def _build_bass():
    nc = bass.Bass("TRN2", target_bir_lowering=False)
    x_d = nc.dram_tensor("x", [N, B], MM_DT, kind="ExternalInput")
    w_d = nc.dram_tensor("w", [B, B], F32, kind="ExternalInput")
    b_d = nc.dram_tensor("b", [B], F32, kind="ExternalInput")
    y_d = nc.dram_tensor("y", [N, B], F32, kind="ExternalOutput")

    with tile.TileContext(nc) as tc:
        with (
            tc.tile_pool(name="const", bufs=1) as const_pool,
            tc.tile_pool(name="xin", bufs=3) as x_pool,
            tc.tile_pool(name="yout", bufs=3) as y_pool,
            tc.tile_pool(name="xT", bufs=3) as xT_pool,
            tc.tile_pool(name="psT", bufs=3, space="PSUM") as psT_pool,
            tc.tile_pool(name="psY", bufs=4, space="PSUM") as psY_pool,
            tc.tile_pool(name="psDummy", bufs=1, space="PSUM") as psD_pool,
        ):
            ident = const_pool.tile([P, P], F32)
            make_identity(nc, ident)
            # float32r identity for the x transposes (produced rounded, as
            # the BIR verifier requires for fp32r matmul operands)
            ident_r = const_pool.tile([P, P], MM_DT)
            nc.vector.tensor_copy(out=ident_r, in_=ident)

            # Wait-staggering: walrus accepts one attached sync wait per
            # instruction (extras are split onto NoOps post-hoc, but the
            # dummy keeps the hot path clean). Absorb the identity (Pool)
            # wait before the first real PE instruction.
            ps_dummy = psD_pool.tile([P, P], F32)
            dummy_inst = nc.tensor.transpose(ps_dummy, ident, ident)

            # bias replicated to all 128 partitions: [128, 512]
            bias_rep = const_pool.tile([P, B], F32)
            nc.sync.dma_start(
                out=bias_rep,
                in_=b_d.ap().unsqueeze(0).partition_broadcast(P),
            )

            # W natural layout: 4 row-blocks side by side -> [128, 4*512]
            # w_nat[:, rj*512:(rj+1)*512] = W[rj*128:(rj+1)*128, :]
            w_nat = const_pool.tile([P, CB * B], F32)
            nc.sync.dma_start(
                out=w_nat.rearrange("p (j c) -> p j c", j=CB),
                in_=w_d.ap().rearrange("(j p) c -> p j c", p=P),
            )

            # wT strips: wT[:, ci*512 + r] (c on partitions) = W[r, ci*128 + c]
            wT = const_pool.tile([P, CB * B], MM_DT)
            for ci in range(CB):
                psT = psT_pool.tile([P, B], F32, tag="ps_t")
                for rj in range(CB):
                    t_inst = nc.tensor.transpose(
                        psT[:, rj * P : (rj + 1) * P],
                        w_nat[:, rj * B + ci * P : rj * B + ci * P + P],
                        ident,
                    )
                    if ci == 0 and rj == 0:
                        add_dep_helper(
                            t_inst.ins, dummy_inst.ins, sync=False,
                            reason="stagger startup waits",
                        )
                nc.scalar.copy(out=wT[:, ci * B : (ci + 1) * B], in_=psT)

            # main loop over 64 token tiles with independent x-load and
            # y-store chunk schedules
            x_bounds = []
            acc = 0
            for g in X_SCHED:
                x_bounds.append((acc, g))
                acc += g
            y_bounds = []
            acc = 0
            for g in Y_SCHED:
                y_bounds.append((acc, g))
                acc += g
            x_next = dict((s, g) for s, g in x_bounds)
            y_next = dict((s, g) for s, g in y_bounds)

            x_big = None
            y_big = None
            x_base = y_base = 0
            for t in range(NT):
                if t in x_next:
                    gx = x_next[t]
                    x_base = t
                    x_big = x_pool.tile([P, gx * B], MM_DT, tag="xbig")
                    nc.sync.dma_start(
                        out=x_big.rearrange("p (g c) -> p g c", g=gx),
                        in_=x_d.ap()[t * P : (t + gx) * P, :].rearrange(
                            "(g p) c -> p g c", p=P
                        ),
                    )
                if t in y_next:
                    gy = y_next[t]
                    y_base = t
                    y_big = y_pool.tile([P, gy * B], F32, tag="ybig")

                xs = x_big[:, (t - x_base) * B : (t - x_base + 1) * B]
                psx = psT_pool.tile([P, B], MM_DT, tag="ps_t")
                for ci in range(CB):
                    t_inst = nc.tensor.transpose(
                        psx[:, ci * P : (ci + 1) * P],
                        xs[:, ci * P : (ci + 1) * P],
                        ident_r,
                    )
                    if t == 0 and ci == 0:
                        add_dep_helper(
                            t_inst.ins, dummy_inst.ins, sync=False,
                            reason="stagger startup waits",
                        )
                xT = xT_pool.tile([P, B], MM_DT)
                # alternate the rounding cast between DVE and ACT to keep
                # the DVE under the DMA roofline
                if t % 2 == 0:
                    nc.vector.tensor_copy(out=xT, in_=psx)
                else:
                    nc.scalar.copy(out=xT, in_=psx)

                psy = psY_pool.tile([P, B], F32)
                for ci in range(CB):
                    nc.tensor.matmul(
                        psy,
                        xT[:, ci * P : (ci + 1) * P],
                        wT[:, ci * B : (ci + 1) * B],
                        start=(ci == 0),
                        stop=(ci == CB - 1),
                    )
                # fused bias add + PSUM->SBUF evacuation
                nc.vector.tensor_add(
                    y_big[:, (t - y_base) * B : (t - y_base + 1) * B],
                    psy,
                    bias_rep,
                )

                if t - y_base == y_next[y_base] - 1:
                    gy = y_next[y_base]
                    nc.sync.dma_start(
                        out=y_d.ap()[y_base * P : (y_base + gy) * P, :].rearrange(
                            "(g p) c -> p g c", p=P
                        ),
                        in_=y_big.rearrange("p (g c) -> p g c", g=gy),
                    )

    return nc


def _split_pe_multiwaits(nc):
    """Hoist extra sync waits off engine instructions onto sequencer NoOps.

    This walrus build supports only a single attached sync wait per
    instruction; codegen fails with "Too many sync wait commands" otherwise.
    A wait-carrying NoOp immediately before the instruction on the same
    sequencer is semantically identical (the sequencer executes in order).
    """
    k = 0
    for f in nc.m.functions:
        for blk in f.blocks:
            out = []
            changed = False
            for inst in blk.instructions:
                si = inst.sync_info
                if si is not None and len(si.on_wait) > 1:
                    waits = list(si.on_wait)
                    for w in waits[:-1]:
                        nop = mybir.InstNoOp(
                            name=f"I-waitsplit-{k}", ins=[], outs=[]
                        )
                        k += 1
                        nop.engine = inst.engine
                        nop.sync_info = mybir.SyncInfo(on_wait=[w], on_update=[])
                        out.append(nop)
                    inst.sync_info = mybir.SyncInfo(
                        on_wait=[waits[-1]], on_update=list(si.on_update)
                    )
                    changed = True
                out.append(inst)
            if changed:
                blk.instructions = out
    return nc


def _get_nc():
    if "nc" not in _CACHE:
        _CACHE["nc"] = _split_pe_multiwaits(_build_bass())
    return _CACHE["nc"]


def _run(inputs, trace=False):
    x = np.ascontiguousarray(np.asarray(inputs["x"], dtype=np.float32))
    w = np.ascontiguousarray(np.asarray(inputs["weight_blocks"], dtype=np.float32))
    bias = np.ascontiguousarray(np.asarray(inputs["bias"], dtype=np.float32))
    assert x.shape == (N, D) and w.shape == (NB, B, B) and bias.shape == (D,)

    nc = _get_nc()
    in_maps = [
        {
            "x": np.ascontiguousarray(x[:, k * B : (k + 1) * B]),
            "w": np.ascontiguousarray(w[k]),
            "b": np.ascontiguousarray(bias[k * B : (k + 1) * B]),
        }
        for k in range(NB)
    ]
    res = run_bass_kernel_spmd(nc, in_maps, core_ids=list(range(NB)), trace=trace)
    y = np.concatenate([res.results[k]["y"] for k in range(NB)], axis=1)
    return np.asarray(y, dtype=np.float32), res


def kernel(**inputs):
    y, _ = _run(inputs, trace=False)
    return y


def kernel_traced(**inputs):
    return _run(inputs, trace=True)


# revision 21
# speedup vs baseline: 1.0672x; 1.0505x over previous
"""Block-diagonal linear y = x @ W_blockdiag.T + bias on 8 TRN2 NeuronCores.

Expert-parallel sharding: core k owns diagonal block k — x[:, 512k:512(k+1)],
weight_blocks[k] (512x512), bias[512k:512(k+1)] — and produces the matching
output column slice y[:, 512k:512(k+1)]. No collectives.

Per-core kernel (Tile framework):
  - load x in staggered chunks (small first chunks for fast pipeline fill,
    2 MiB steady-state DMAs)
  - PE-transpose each [128,128] sub-block of an x tile into PSUM (float32r,
    1.5 cyc/row), evacuate as one [128, 512] strip to SBUF with a rounding
    cast -> xT blocks [c=128, n=128] in float32r
  - 4 accumulating matmuls per token tile: stationary lhsT = xT block,
    moving rhs = W.T strip [c=128, r=512], float32r (1 cyc/row)
  - bias add fused into the PSUM->SBUF evacuation on DVE
  - store y in staggered chunks (small last chunks to shorten the tail)
"""

import os
import sys

import numpy as np

for _p in ("/opt/trn_rl_repo", "/root/.axon_site/_ro/trn_rl_repo"):
    if os.path.isdir(_p) and _p not in sys.path:
        sys.path.insert(0, _p)

import concourse.bass as bass
import concourse.mybir as mybir
import concourse.tile as tile
from concourse.masks import make_identity
from concourse.bass_utils import run_bass_kernel_spmd
from concourse.tile_rust import add_dep_helper

# Problem shape (hardcoded per spec nn_BlockDiagLinear_19490561590005)
N = 8192          # tokens
D = 4096          # model dim
NB = 8            # diagonal blocks == number of cores
B = 512           # block size (rows == cols)
P = 128           # SBUF partitions
CB = B // P       # 4 contraction chunks of 128
NT = N // P       # 64 token tiles

F32 = mybir.dt.float32
# float32r: 1 cycle/row on the PE for free dim >= 256 (vs 4 for float32)
MM_DT = getattr(mybir.dt, os.environ.get("BD_MM_DT", "float32r"))

# token tiles per DMA chunk. Within a chunk the DMA uses a "(p g) c"
# row<->partition mapping: partition p holds g consecutive DRAM rows, so
# every descriptor is a fully contiguous g*2KB stripe (max DMA efficiency).
# That mapping forces x-load and y-store chunk boundaries to coincide.
# Small first chunk = fast pipeline fill; small last chunks = short tail.
SCHED = [4, 8, 8, 8, 8, 8, 8, 8, 2, 2]
assert sum(SCHED) == NT

_CACHE = {}


def _build_bass():
    nc = bass.Bass("TRN2", target_bir_lowering=False)
    x_d = nc.dram_tensor("x", [N, B], MM_DT, kind="ExternalInput")
    w_d = nc.dram_tensor("w", [B, B], F32, kind="ExternalInput")
    b_d = nc.dram_tensor("b", [B], F32, kind="ExternalInput")
    y_d = nc.dram_tensor("y", [N, B], F32, kind="ExternalOutput")

    with tile.TileContext(nc) as tc:
        with (
            tc.tile_pool(name="const", bufs=1) as const_pool,
            tc.tile_pool(name="xin", bufs=3) as x_pool,
            tc.tile_pool(name="yout", bufs=3) as y_pool,
            tc.tile_pool(name="xT", bufs=3) as xT_pool,
            tc.tile_pool(name="psT", bufs=3, space="PSUM") as psT_pool,
            tc.tile_pool(name="psY", bufs=4, space="PSUM") as psY_pool,
            tc.tile_pool(name="psDummy", bufs=1, space="PSUM") as psD_pool,
        ):
            ident = const_pool.tile([P, P], F32)
            make_identity(nc, ident)
            # float32r identity for the x transposes (produced rounded, as
            # the BIR verifier requires for fp32r matmul operands)
            ident_r = const_pool.tile([P, P], MM_DT)
            nc.vector.tensor_copy(out=ident_r, in_=ident)

            # Wait-staggering: walrus accepts one attached sync wait per
            # instruction (extras are split onto NoOps post-hoc, but the
            # dummy keeps the hot path clean). Absorb the identity (Pool)
            # wait before the first real PE instruction.
            ps_dummy = psD_pool.tile([P, P], F32)
            dummy_inst = nc.tensor.transpose(ps_dummy, ident, ident)

            # bias replicated to all 128 partitions: [128, 512]
            bias_rep = const_pool.tile([P, B], F32)
            nc.sync.dma_start(
                out=bias_rep,
                in_=b_d.ap().unsqueeze(0).partition_broadcast(P),
            )

            # W natural layout: 4 row-blocks side by side -> [128, 4*512]
            # w_nat[:, rj*512:(rj+1)*512] = W[rj*128:(rj+1)*128, :]
            w_nat = const_pool.tile([P, CB * B], F32)
            nc.sync.dma_start(
                out=w_nat.rearrange("p (j c) -> p j c", j=CB),
                in_=w_d.ap().rearrange("(j p) c -> p j c", p=P),
            )

            # wT strips: wT[:, ci*512 + r] (c on partitions) = W[r, ci*128 + c]
            wT = const_pool.tile([P, CB * B], MM_DT)
            for ci in range(CB):
                psT = psT_pool.tile([P, B], F32, tag="ps_t")
                for rj in range(CB):
                    t_inst = nc.tensor.transpose(
                        psT[:, rj * P : (rj + 1) * P],
                        w_nat[:, rj * B + ci * P : rj * B + ci * P + P],
                        ident,
                    )
                    if ci == 0 and rj == 0:
                        add_dep_helper(
                            t_inst.ins, dummy_inst.ins, sync=False,
                            reason="stagger startup waits",
                        )
                nc.scalar.copy(out=wT[:, ci * B : (ci + 1) * B], in_=psT)

            # main loop over 64 token tiles, chunked per SCHED
            chunk_of = {}
            acc = 0
            for g in SCHED:
                chunk_of[acc] = g
                acc += g

            x_big = None
            y_big = None
            base = 0
            for t in range(NT):
                if t in chunk_of:
                    g = chunk_of[t]
                    base = t
                    x_big = x_pool.tile([P, g * B], MM_DT, tag="xbig")
                    nc.sync.dma_start(
                        out=x_big.rearrange("p (g c) -> p g c", g=g),
                        in_=x_d.ap()[t * P : (t + g) * P, :].rearrange(
                            "(p g) c -> p g c", g=g
                        ),
                    )
                    y_big = y_pool.tile([P, g * B], F32, tag="ybig")

                xs = x_big[:, (t - base) * B : (t - base + 1) * B]
                psx = psT_pool.tile([P, B], MM_DT, tag="ps_t")
                for ci in range(CB):
                    t_inst = nc.tensor.transpose(
                        psx[:, ci * P : (ci + 1) * P],
                        xs[:, ci * P : (ci + 1) * P],
                        ident_r,
                    )
                    if t == 0 and ci == 0:
                        add_dep_helper(
                            t_inst.ins, dummy_inst.ins, sync=False,
                            reason="stagger startup waits",
                        )
                xT = xT_pool.tile([P, B], MM_DT, bufs=4)
                # alternate the rounding cast between DVE and ACT to keep
                # the DVE under the DMA roofline
                if t % 2 == 0:
                    nc.vector.tensor_copy(out=xT, in_=psx)
                else:
                    nc.scalar.copy(out=xT, in_=psx)

                psy = psY_pool.tile([P, B], F32)
                for ci in range(CB):
                    nc.tensor.matmul(
                        psy,
                        xT[:, ci * P : (ci + 1) * P],
                        wT[:, ci * B : (ci + 1) * B],
                        start=(ci == 0),
                        stop=(ci == CB - 1),
                    )
                # fused bias add + PSUM->SBUF evacuation
                nc.vector.tensor_add(
                    y_big[:, (t - base) * B : (t - base + 1) * B],
                    psy,
                    bias_rep,
                )

                if t - base == chunk_of[base] - 1:
                    g = chunk_of[base]
                    nc.sync.dma_start(
                        out=y_d.ap()[base * P : (base + g) * P, :].rearrange(
                            "(p g) c -> p g c", g=g
                        ),
                        in_=y_big.rearrange("p (g c) -> p g c", g=g),
                    )

    return nc


def _split_pe_multiwaits(nc):
    """Hoist extra sync waits off engine instructions onto sequencer NoOps.

    This walrus build supports only a single attached sync wait per
    instruction; codegen fails with "Too many sync wait commands" otherwise.
    A wait-carrying NoOp immediately before the instruction on the same
    sequencer is semantically identical (the sequencer executes in order).
    """
    k = 0
    for f in nc.m.functions:
        for blk in f.blocks:
            out = []
            changed = False
            for inst in blk.instructions:
                si = inst.sync_info
                if si is not None and len(si.on_wait) > 1:
                    waits = list(si.on_wait)
                    for w in waits[:-1]:
                        nop = mybir.InstNoOp(
                            name=f"I-waitsplit-{k}", ins=[], outs=[]
                        )
                        k += 1
                        nop.engine = inst.engine
                        nop.sync_info = mybir.SyncInfo(on_wait=[w], on_update=[])
                        out.append(nop)
                    inst.sync_info = mybir.SyncInfo(
                        on_wait=[waits[-1]], on_update=list(si.on_update)
                    )
                    changed = True
                out.append(inst)
            if changed:
                blk.instructions = out
    return nc


def _get_nc():
    if "nc" not in _CACHE:
        _CACHE["nc"] = _split_pe_multiwaits(_build_bass())
    return _CACHE["nc"]


def _run(inputs, trace=False):
    x = np.ascontiguousarray(np.asarray(inputs["x"], dtype=np.float32))
    w = np.ascontiguousarray(np.asarray(inputs["weight_blocks"], dtype=np.float32))
    bias = np.ascontiguousarray(np.asarray(inputs["bias"], dtype=np.float32))
    assert x.shape == (N, D) and w.shape == (NB, B, B) and bias.shape == (D,)

    nc = _get_nc()
    in_maps = [
        {
            "x": np.ascontiguousarray(x[:, k * B : (k + 1) * B]),
            "w": np.ascontiguousarray(w[k]),
            "b": np.ascontiguousarray(bias[k * B : (k + 1) * B]),
        }
        for k in range(NB)
    ]
    res = run_bass_kernel_spmd(nc, in_maps, core_ids=list(range(NB)), trace=trace)
    y = np.concatenate([res.results[k]["y"] for k in range(NB)], axis=1)
    return np.asarray(y, dtype=np.float32), res


def kernel(**inputs):
    y, _ = _run(inputs, trace=False)
    return y


def kernel_traced(**inputs):
    return _run(inputs, trace=True)
